# revision 3
# baseline (speedup 1.0000x reference)
"""Trainium2 Bass kernel for nn_Curve_64690797412573 (histogram_binning).

res[b,c,h,w] = curve[b,c, idx] with idx = clip(img*255,0,255).int,
curve = global-minmax-normalized cumsum(sigmoid(rr/gr/br)).

Strategy (8 NeuronCores, data-parallel over batch):
  core k handles batches (2k, 2k+1) x 3 channels = 6 images of [1024,1024].
  Since curve is a cumsum, res = sum_j w_j * [j <= 255*x] with
  w_j = sigmoid(a)_j/(M-m) and the j=0 weight also carrying -m/(M-m)
  (that term is always on). The 256-term masked sum for 128 pixels (one
  per partition) is ONE fused DVE scalar_tensor_tensor instruction:
      scr = (jtab is_le v_col) * wtab ; acc_col = sum(scr)
  Curves are computed on-device per shard (rr/gr/br replicated, shard's
  two batch rows swapped into rows 0,1 - global min/max is order-invariant).
"""
import sys
sys.path.insert(0, '/opt/trn_rl_repo')
import numpy as np

_CACHE = {}

B, C, H, W = 16, 3, 1024, 1024
NBINS = 256
N_CORES = 8
N_IMAGES = 6          # per core: 2 batches x 3 channels
TILE_COLS = 512


def _build():
    import concourse.bacc as bacc
    import concourse.mybir as mybir
    from concourse.tile import TileContext

    F32 = mybir.dt.float32
    I32 = mybir.dt.int32
    ALU = mybir.AluOpType
    AXT = mybir.AxisListType

    nc = bacc.Bacc('TRN2', target_bir_lowering=False, debug=False,
                   num_devices=N_CORES)
    rows_total = N_IMAGES * H
    img_d = nc.dram_tensor('img', [rows_total, W], F32, kind='ExternalInput')
    ch_d = [nc.dram_tensor(n, [B, NBINS], F32, kind='ExternalInput')
            for n in ('rr', 'gr', 'br')]
    out_d = nc.dram_tensor('out', [rows_total, W], F32, kind='ExternalOutput')

    row_blocks = H // 128
    col_tiles = W // TILE_COLS

    with TileContext(nc) as tc:
        with tc.tile_pool(name='tab', bufs=1) as tabp, \
             tc.tile_pool(name='work', bufs=3) as workp, \
             tc.tile_pool(name='wt', bufs=2) as wtp, \
             tc.tile_pool(name='dram', bufs=1, space='DRAM') as dramp, \
             tc.tile_pool(name='small', bufs=1) as smallp:

            # jtab: 0..255 (fp32) in every partition
            jtab_i = tabp.tile([128, NBINS], I32, tag='jtabi')
            nc.gpsimd.iota(jtab_i[:, :], pattern=[[1, NBINS]], base=0,
                           channel_multiplier=0)
            jtab = tabp.tile([128, NBINS], F32, tag='jtab')
            nc.vector.tensor_copy(jtab[:, :], jtab_i[:, :])

            # curves per channel
            wch = []
            for ci in range(3):
                raw = smallp.tile([B, NBINS], F32, tag=f'raw{ci}')
                nc.sync.dma_start(raw[:, :], ch_d[ci].ap())
                sig = smallp.tile([B, NBINS], F32, tag=f'sig{ci}')
                nc.scalar.activation(sig[:, :], raw[:, :],
                                     mybir.ActivationFunctionType.Sigmoid)
                zero = smallp.tile([B, NBINS], F32, tag=f'zz{ci}')
                nc.vector.memset(zero[:, :], 0.0)
                cum = smallp.tile([B, NBINS], F32, tag=f'cum{ci}')
                nc.vector.tensor_tensor_scan(
                    cum[:, :], sig[:, :], zero[:, :], 0.0, ALU.add, ALU.add)
                # global min/max (cross-partition reduce: only max -> negate)
                negc = smallp.tile([B, NBINS], F32, tag=f'negc{ci}')
                nc.vector.tensor_scalar_mul(negc[:, :], cum[:, :], -1.0)
                gminneg = smallp.tile([1, 1], F32, tag=f'gminneg{ci}')
                gmax = smallp.tile([1, 1], F32, tag=f'gmax{ci}')
                nc.gpsimd.tensor_reduce(gminneg[:, :], negc[:, :], AXT.XYZWC,
                                        ALU.max)
                nc.gpsimd.tensor_reduce(gmax[:, :], cum[:, :], AXT.XYZWC,
                                        ALU.max)
                span = smallp.tile([1, 1], F32, tag=f'span{ci}')
                nc.vector.tensor_tensor(span[:, :], gmax[:, :], gminneg[:, :],
                                        ALU.add)
                r = smallp.tile([1, 1], F32, tag=f'r{ci}')
                nc.vector.reciprocal(r[:, :], span[:, :])
                mr = smallp.tile([1, 1], F32, tag=f'mr{ci}')
                nc.vector.tensor_tensor(mr[:, :], gminneg[:, :], r[:, :],
                                        ALU.mult)
                sigd = dramp.tile([B, NBINS], F32, tag=f'sigd{ci}')
                nc.sync.dma_start(sigd[:, :], sig[:, :])
                rd = dramp.tile([1, 1], F32, tag=f'rd{ci}')
                mrd = dramp.tile([1, 1], F32, tag=f'mrd{ci}')
                nc.sync.dma_start(rd[:, :], r[:, :])
                nc.sync.dma_start(mrd[:, :], mr[:, :])
                wch.append((sigd, rd, mrd))

            for i in range(N_IMAGES):
                b_loc = i // 3
                ci = i % 3
                sigd, rd, mrd = wch[ci]
                wraw = wtp.tile([128, NBINS], F32, tag='wraw')
                nc.sync.dma_start(
                    wraw[:, :],
                    sigd[b_loc:b_loc + 1, :].broadcast_to([128, NBINS]))
                rb = wtp.tile([128, 1], F32, tag='rb')
                mrb = wtp.tile([128, 1], F32, tag='mrb')
                nc.sync.dma_start(rb[:, :], rd[0:1, 0:1].broadcast_to([128, 1]))
                nc.sync.dma_start(mrb[:, :],
                                  mrd[0:1, 0:1].broadcast_to([128, 1]))
                wtab = wtp.tile([128, NBINS], F32, tag='wtab')
                nc.vector.tensor_scalar_mul(wtab[:, :], wraw[:, :], rb[:, 0:1])
                nc.vector.tensor_tensor(wtab[:, 0:1], wtab[:, 0:1],
                                        mrb[:, 0:1], ALU.add)
                for rb_i in range(row_blocks):
                    for ct in range(col_tiles):
                        r0 = (i * row_blocks + rb_i) * 128
                        x = workp.tile([128, TILE_COLS], F32, tag='x')
                        nc.sync.dma_start(
                            x[:, :],
                            img_d.ap()[r0:r0 + 128,
                                       ct * TILE_COLS:(ct + 1) * TILE_COLS])
                        v = workp.tile([128, TILE_COLS], F32, tag='v')
                        nc.vector.tensor_scalar_mul(v[:, :], x[:, :], 255.0)
                        acc = workp.tile([128, TILE_COLS], F32, tag='acc')
                        for n in range(TILE_COLS):
                            scr = workp.tile([128, NBINS], F32, tag='scr')
                            nc.vector.scalar_tensor_tensor(
                                scr[:, :], jtab[:, :], v[:, n:n + 1],
                                wtab[:, :], ALU.is_le, ALU.mult,
                                accum_out=acc[:, n:n + 1])
                        nc.sync.dma_start(
                            out_d.ap()[r0:r0 + 128,
                                       ct * TILE_COLS:(ct + 1) * TILE_COLS],
                            acc[:, :])
    nc.compile()
    return nc


def _get_nc():
    if 'nc' not in _CACHE:
        _CACHE['nc'] = _build()
    return _CACHE['nc']


class _Exec:
    """Compile the Bass module to a PJRT executable once; reuse across calls.

    Mirrors concourse.bass2jax.run_bass_via_pjrt's multi-core path, but keeps
    the jitted function (and device placement) cached so repeated kernel()
    invocations skip retracing/recompilation.
    """

    def __init__(self, nc):
        import jax
        import numpy as _np
        import concourse.mybir as mybir
        from jax.sharding import Mesh, PartitionSpec
        from jax.experimental.shard_map import shard_map
        from concourse.bass2jax import (_bass_exec_p, install_neuronx_cc_hook,
                                        partition_id_tensor)
        install_neuronx_cc_hook()
        self.jax = jax
        partition_name = (nc.partition_id_tensor.name
                          if nc.partition_id_tensor else None)
        in_names, out_names, out_avals, zero_outs = [], [], [], []
        for alloc in nc.m.functions[0].allocations:
            if not isinstance(alloc, mybir.MemoryLocationSet):
                continue
            name = alloc.memorylocations[0].name
            if alloc.kind == 'ExternalInput':
                if name != partition_name:
                    in_names.append(name)
            elif alloc.kind == 'ExternalOutput':
                out_names.append(name)
                shape = tuple(alloc.tensor_shape)
                dtype = mybir.dt.np(alloc.dtype)
                out_avals.append(jax.core.ShapedArray(shape, dtype))
                zero_outs.append(_np.zeros(shape, dtype))
        self.in_names = in_names
        self.out_names = out_names
        self.out_avals = out_avals
        self.zero_outs = zero_outs
        n_params = len(in_names)
        self.n_params = n_params
        n_outs = len(out_avals)
        all_in_names = in_names + out_names
        if partition_name is not None:
            all_in_names.append(partition_name)

        def _body(*args):
            operands = list(args)
            if partition_name is not None:
                operands.append(partition_id_tensor())
            outs = _bass_exec_p.bind(
                *operands,
                out_avals=tuple(out_avals),
                in_names=tuple(all_in_names),
                out_names=tuple(out_names),
                lowering_input_output_aliases=(),
                sim_require_finite=True,
                sim_require_nnan=True,
                nc=nc,
            )
            return tuple(outs)

        devices = jax.devices()[:N_CORES]
        mesh = Mesh(_np.asarray(devices), ('core',))
        in_specs = (PartitionSpec('core'),) * (n_params + n_outs)
        out_specs = (PartitionSpec('core'),) * len(out_names)
        self.fn = jax.jit(
            shard_map(_body, mesh=mesh, in_specs=in_specs,
                      out_specs=out_specs, check_rep=False),
            keep_unused=True,
        )

    def run(self, in_maps):
        import numpy as _np
        per_core = [[_np.asarray(m[name]) for name in self.in_names]
                    for m in in_maps]
        concat_in = [
            _np.concatenate([per_core[c][i] for c in range(N_CORES)], axis=0)
            for i in range(self.n_params)
        ]
        concat_zeros = [
            _np.zeros((N_CORES * z.shape[0], *z.shape[1:]), z.dtype)
            for z in self.zero_outs
        ]
        outs = self.fn(*concat_in, *concat_zeros)
        self.jax.block_until_ready(outs)
        return [
            {name: _np.asarray(outs[i]).reshape(
                N_CORES, *self.out_avals[i].shape)[c]
             for i, name in enumerate(self.out_names)}
            for c in range(N_CORES)
        ]


def _get_exec():
    if 'exec' not in _CACHE:
        _CACHE['exec'] = _Exec(_get_nc())
    return _CACHE['exec']


def _shard_inputs(img_low, rr, gr, br):
    img_low = np.ascontiguousarray(img_low, dtype=np.float32)
    in_maps = []
    for k in range(N_CORES):
        b0, b1 = 2 * k, 2 * k + 1
        shard = np.ascontiguousarray(
            img_low[b0:b1 + 1].reshape(N_IMAGES * H, W))
        perm = list(range(B))
        perm[0], perm[b0] = perm[b0], perm[0]
        perm[1], perm[b1] = perm[b1], perm[1]

        def reorder(a):
            return np.ascontiguousarray(a, dtype=np.float32)[perm]

        in_maps.append({'img': shard, 'rr': reorder(rr), 'gr': reorder(gr),
                        'br': reorder(br)})
    return in_maps


def kernel(img_low, rr, gr, br):
    ex = _get_exec()
    in_maps = _shard_inputs(img_low, rr, gr, br)
    results = ex.run(in_maps)
    out = np.empty((B, C, H, W), dtype=np.float32)
    for k in range(N_CORES):
        out[2 * k:2 * k + 2] = results[k]['out'].reshape(2, C, H, W)
    return out
